# revision 10
# baseline (speedup 1.0000x reference)
"""2-layer GCN (GCNConv x2) on 8 TRN2 NeuronCores.

Strategy: partition destination nodes across cores (12500/core, padded to
12544 = 98 windows x 128 lanes). Per layer: local X@W matmul -> pre-scale by
D^-1/2 -> AllGather a bf16 message table -> per-edge gather via dma_gather
(int16 idx, 4 residue banks of 64B-strided rows) -> one-hot scatter matmul
into persistent PSUM window accumulators -> epilogue (scale, bias, act).
norm factors: out[d] = dis[d] * sum_e dis[src] * msg[src]  (self-loops are
plain edges), so no per-edge norm work is needed.
"""
import sys
import numpy as np
import ml_dtypes

from concourse import bass, mybir, tile, bacc, bass_utils
from concourse.library_config import mlp
from concourse.bass import exact_div

NC = 8
NN = 100000
NPC = 12500
SLOTS = 12544          # 98 windows * 128 lanes
W = 98
P = 128
F0, F1, F2 = 128, 16, 8
NQ = 4                 # residue banks (table row = 64B, stride = 256B)
MAXT_CALL = 31         # tiles per dma_gather call (<= 3968 idx, ring limit)

BF16 = mybir.dt.bfloat16
F32 = mybir.dt.float32
I16 = mybir.dt.int16


def _host_prep(x, edge_index, W1, b1, W2, b2):
    """Shard + schedule. Returns in_maps (list of dicts) and schedule."""
    s = edge_index[0].astype(np.int64)
    d = edge_index[1].astype(np.int64)
    loops = np.arange(NN, dtype=np.int64)
    s = np.concatenate([s, loops])
    d = np.concatenate([d, loops])

    deg = np.bincount(d, minlength=NN).astype(np.float32)  # includes loops

    cd = d // NPC
    jd = d - cd * NPC
    wd = jd >> 7
    pd = jd & 127
    cs = s // NPC
    js = s - cs * NPC
    g = cs * SLOTS + (js & 127) * W + (js >> 7)
    q = g & (NQ - 1)
    idx16 = (g >> 2).astype(np.int16)

    # group key: (core, q, w)
    key = (cd * NQ + q) * W + wd
    order = np.argsort(key, kind="stable")
    key_s = key[order]
    idx16_s = idx16[order]
    pd_s = pd[order]

    counts = np.bincount(key, minlength=NC * NQ * W).reshape(NC, NQ, W)
    T = np.maximum(1, -(-counts.max(axis=0) // 128))      # [NQ, W] tiles
    seg_slots = T * 128
    seg_start = np.concatenate([[0], np.cumsum(seg_slots.ravel())]).astype(np.int64)
    ntot = int(seg_start[-1])

    # within-group rank of each sorted edge
    grp_first = np.concatenate([[0], np.cumsum(np.bincount(key_s, minlength=NC * NQ * W))])
    rank = np.arange(len(key_s)) - grp_first[key_s]
    core_of = key_s // (NQ * W)
    seg_of = key_s % (NQ * W)
    pos = seg_start[seg_of] + rank

    idxflat = np.zeros((NC, ntot), dtype=np.int16)
    dlflat = np.full((NC, ntot), -1.0, dtype=np.float32)
    idxflat[core_of, pos] = idx16_s
    dlflat[core_of, pos] = pd_s.astype(np.float32)

    # images
    nt16 = ntot // 16
    ntt = ntot // 128
    idximg = np.zeros((NC, 128, nt16), dtype=np.int16)
    dlimg = np.zeros((NC, 128, ntt), dtype=np.float32)
    for c in range(NC):
        w16 = np.ascontiguousarray(idxflat[c].reshape(nt16, 16).T)
        for r in range(2 * NQ):   # replicate for queue core pairs 0..NQ-1
            idximg[c, r * 16:(r + 1) * 16] = w16
        dlimg[c] = dlflat[c].reshape(ntt, 128).T

    # per-tile (w, start, stop) + call schedule (within one q)
    tile_w, tile_st, tile_sp, calls = [], [], [], []
    gt = 0
    for qq in range(NQ):
        q_start = gt
        for ww in range(W):
            t = int(T[qq, ww])
            for k in range(t):
                tile_w.append(ww)
                tile_st.append(qq == 0 and k == 0)
                tile_sp.append(qq == NQ - 1 and k == t - 1)
            gt += t
        # split this q's tile range into calls
        a = q_start
        while a < gt:
            n = min(MAXT_CALL, gt - a)
            calls.append((a, n, qq))
            a += n
    ntiles = gt
    assert ntiles * 128 == ntot

    # dis arrays [128, 98] (exact f32 on host)
    dis_full = (1.0 / np.sqrt(deg)).astype(np.float32)
    degs = []
    for c in range(NC):
        dc = np.ones(SLOTS, dtype=np.float32)
        dc[:NPC] = dis_full[c * NPC:(c + 1) * NPC]
        degs.append(np.ascontiguousarray(dc.reshape(W, 128).T))

    in_maps = []
    for c in range(NC):
        xt = np.zeros((128, SLOTS), dtype=np.float32)
        xt[:, :NPC] = x[c * NPC:(c + 1) * NPC].T
        in_maps.append({
            "xT": xt,
            "idximg": idximg[c],
            "dlimg": dlimg[c],
            "dis": degs[c],
            "W1": W1.astype(np.float32),
            "W2": W2.astype(np.float32),
            "b1rep": np.tile(b1.astype(np.float32)[None, :], (128, 1)),
            "b2rep": np.tile(b2.astype(np.float32)[None, :], (128, 1)),
            "iota": np.tile(np.arange(128, dtype=np.float32)[None, :], (128, 1)),
        })
    sched = dict(ntot=ntot, ntiles=ntiles, tile_w=tile_w, tile_st=tile_st,
                 tile_sp=tile_sp, calls=calls)
    return in_maps, sched


def _gather(gp, out_ap, in_ap, idxs_ap, num_idxs, elem_size, elem_step, qnum=0):
    stride_bytes = elem_step * mybir.dt.size(in_ap.dtype)
    return gp.add_instruction(mybir.InstDMAGatherAnt(
        name=gp.bass.get_next_instruction_name(),
        ins=[*gp.lower_ap_dma(in_ap, for_custom_bir_dma=True), gp.lower_ap(idxs_ap),
             gp.lower_val_access(gp.to_reg(num_idxs))],
        outs=[gp.lower_ap(out_ap)],
        transpose=False, num_idxs=num_idxs, elem_size=elem_size,
        stride_bytes_256=exact_div(stride_bytes, 256), gen_mode=0,
        single_packet=False, queue_num=qnum, sbuf_tokens_per_rank=0,
        sbuf_free_dim_per_rank=0, sbuf_free_dim_pad_per_rank=0, sbuf_byte_offset=0))


def build(sched, n_queues=2):
    from concourse.masks import make_identity
    nc = bacc.Bacc("TRN2", target_bir_lowering=False, debug=False, num_devices=NC,
                   num_swdge_queues=n_queues)
    ntot, ntiles = sched["ntot"], sched["ntiles"]
    tile_w, tile_st, tile_sp = sched["tile_w"], sched["tile_st"], sched["tile_sp"]
    calls = sched["calls"]

    xT_d = nc.dram_tensor("xT", [128, SLOTS], F32, kind="ExternalInput")
    idx_d = nc.dram_tensor("idximg", [128, ntot // 16], I16, kind="ExternalInput")
    dl_d = nc.dram_tensor("dlimg", [128, ntot // 128], F32, kind="ExternalInput")
    dis_d = nc.dram_tensor("dis", [128, W], F32, kind="ExternalInput")
    W1_d = nc.dram_tensor("W1", [F0, F1], F32, kind="ExternalInput")
    W2_d = nc.dram_tensor("W2", [F1, F2], F32, kind="ExternalInput")
    b1_d = nc.dram_tensor("b1rep", [128, F1], F32, kind="ExternalInput")
    b2_d = nc.dram_tensor("b2rep", [128, F2], F32, kind="ExternalInput")
    io_d = nc.dram_tensor("iota", [128, 128], F32, kind="ExternalInput")
    out_d = nc.dram_tensor("out", [128, W * F2], F32, kind="ExternalOutput")
    hs_d = nc.dram_tensor("hs_dump", [128, W * F1], F32, kind="ExternalOutput")
    x1_d = nc.dram_tensor("x1_dump", [128, W * F1], F32, kind="ExternalOutput")

    with tile.TileContext(nc) as tc:
        nc.gpsimd.load_library(mlp)
        with (tc.tile_pool(name="sb", bufs=1) as sb,
              tc.tile_pool(name="gp", bufs=3) as gpool,
              tc.tile_pool(name="hp", bufs=4) as hpool,
              tc.tile_pool(name="ep", bufs=2) as epool,
              tc.tile_pool(name="ps", bufs=1, space="PSUM") as psp,
              tc.tile_pool(name="psm", bufs=1, space="PSUM") as psm,
              tc.tile_pool(name="dr", bufs=1, space="DRAM") as dr):
            # ---- static loads
            xT = sb.tile([128, SLOTS], F32)
            nc.sync.dma_start(out=xT[:], in_=xT_d.ap())
            IDX = sb.tile([128, ntot // 16], I16)
            nc.sync.dma_start(out=IDX[:], in_=idx_d.ap())
            DL = sb.tile([128, ntot // 128], F32)
            nc.sync.dma_start(out=DL[:], in_=dl_d.ap())
            DIS = sb.tile([128, W], F32)
            nc.sync.dma_start(out=DIS[:], in_=dis_d.ap())
            W1s = sb.tile([F0, F1], F32)
            nc.sync.dma_start(out=W1s[:], in_=W1_d.ap())
            W2s = sb.tile([F1, F2], F32)
            nc.sync.dma_start(out=W2s[:], in_=W2_d.ap())
            B1 = sb.tile([128, F1], F32)
            nc.sync.dma_start(out=B1[:], in_=b1_d.ap())
            B2 = sb.tile([128, F2], F32)
            nc.sync.dma_start(out=B2[:], in_=b2_d.ap())
            IOTA = sb.tile([128, 128], F32)
            nc.sync.dma_start(out=IOTA[:], in_=io_d.ap())
            IDENT = sb.tile([128, 128], F32)
            make_identity(nc, IDENT[:])


            STG = [sb.tile([128, W * 16], F32, tag=f"stg{i}", name=f"stg{i}") for i in range(2)]
            OUT = sb.tile([128, W * F2], F32)

            def edge_phase(layer):
                """gather from tbl{layer}, scatter into PSUM slots, epilogue."""
                tbl = dr.tile([NC * 128, W * 16], F32, tag=f"tbl{layer}")
                bnc = dr.tile([128, W * 16], F32, tag=f"bnc{layer}")
                nc.sync.dma_start(out=bnc[:], in_=STG[layer][:])
                cc = nc.gpsimd.collective_compute(
                    "AllGather", mybir.AluOpType.bypass,
                    replica_groups=[list(range(NC))],
                    ins=[bnc[:].opt()], outs=[tbl[:].opt()])
                tflat = tbl[:].rearrange("a b -> (a b)").rearrange(
                    "(m q e) -> q m e", q=NQ, e=16)
                ps = [psp.tile([128, 512], F32, tag=f"ps{i}", name=f"ps{i}") for i in range(4)]
                for t_ in ps:
                    nc.vector.memset(t_[:], 0.0)

                def pslot(w):
                    return ps[w // 32][:, (w % 32) * 16:(w % 32) * 16 + 16]

                from concourse.tile_rust import add_dep_helper
                for ci, (t0, nt, qq) in enumerate(calls):
                    g = gpool.tile([128, MAXT_CALL, 16], F32, tag="g")
                    gi = _gather(nc.gpsimd, g[:, 0:nt, :], tflat[qq],
                                 IDX[:, t0 * 8:(t0 + nt) * 8], nt * 128, 16, 64,
                                 qnum=ci % 2)
                    add_dep_helper(gi.ins, cc.ins, sync=True,
                                   reason="gather reads allgathered table")
                    for k in range(nt):
                        t = t0 + k
                        ww = tile_w[t]
                        H = hpool.tile([128, 128], F32, tag="h")
                        nc.vector.tensor_scalar(
                            out=H[:], in0=IOTA[:], scalar1=DL[:, t:t + 1],
                            scalar2=None, op0=mybir.AluOpType.is_equal)
                        nc.tensor.matmul(out=pslot(ww), lhsT=H[:],
                                         rhs=g[:, k, :],
                                         start=False, stop=tile_sp[t],
                                         skip_group_check=True)
                        if tile_sp[t]:
                            dis_w = DIS[:, ww:ww + 1]
                            if layer == 0:
                                t1 = epool.tile([128, F1], F32, tag="t1")
                                nc.vector.tensor_scalar(out=t1[:], in0=pslot(ww),
                                                        scalar1=dis_w, scalar2=None,
                                                        op0=mybir.AluOpType.mult)
                                t2 = epool.tile([128, F1], F32, tag="t2")
                                nc.vector.tensor_tensor(out=t2[:], in0=t1[:], in1=B1[:],
                                                        op=mybir.AluOpType.add)
                                t3 = epool.tile([128, F1], F32, tag="t3")
                                nc.scalar.activation(t3[:], t2[:],
                                                     mybir.ActivationFunctionType.Relu)
                                nc.vector.tensor_scalar(
                                    out=STG[1][:, ww * 16:ww * 16 + 16], in0=t3[:],
                                    scalar1=dis_w, scalar2=None,
                                    op0=mybir.AluOpType.mult)
                            else:
                                y = epool.tile([128, F1], F32, tag="y")
                                nc.vector.tensor_scalar(out=y[:], in0=pslot(ww),
                                                        scalar1=dis_w, scalar2=None,
                                                        op0=mybir.AluOpType.mult)
                                pt = psm.tile([F1, 128], F32, tag="pt")
                                nc.tensor.transpose(out=pt[:], in_=y[:],
                                                    identity=IDENT[:])
                                s2t = epool.tile([F1, 128], F32, tag="s2t")
                                nc.scalar.copy(s2t[:], pt[:])
                                po = psm.tile([128, F2], F32, tag="po")
                                nc.tensor.matmul(out=po[:], lhsT=s2t[:], rhs=W2s[:],
                                                 start=True, stop=True)
                                o1 = epool.tile([128, F2], F32, tag="o1")
                                nc.vector.tensor_tensor(out=o1[:], in0=po[:], in1=B2[:],
                                                        op=mybir.AluOpType.add)
                                nc.scalar.activation(
                                    OUT[:, ww * F2:(ww + 1) * F2], o1[:],
                                    mybir.ActivationFunctionType.Sigmoid)

            # ---- layer 1 local matmul + scale -> staging table
            nc.vector.memset(STG[0][:], 0.0)
            nc.vector.memset(STG[1][:], 0.0)
            for w in range(W):
                pm = psm.tile([128, F1], F32, tag="mm1")
                nc.tensor.matmul(out=pm[:], lhsT=xT[:, w * 128:(w + 1) * 128],
                                 rhs=W1s[:], start=True, stop=True)
                nc.vector.tensor_scalar(out=STG[0][:, w * 16:w * 16 + 16], in0=pm[:],
                                        scalar1=DIS[:, w:w + 1], scalar2=None,
                                        op0=mybir.AluOpType.mult)
            nc.sync.dma_start(out=x1_d.ap(), in_=STG[0][:])
            edge_phase(0)
            edge_phase(1)
            nc.sync.dma_start(out=out_d.ap(), in_=OUT[:])
            nc.sync.dma_start(out=hs_d.ap(), in_=STG[1][:])
    nc.compile()
    return nc


_CACHE = {}


def kernel(x, edge_index, W1, b1, W2, b2):
    x = np.asarray(x, dtype=np.float32)
    edge_index = np.asarray(edge_index)
    in_maps, sched = _host_prep(x, edge_index, np.asarray(W1), np.asarray(b1),
                                np.asarray(W2), np.asarray(b2))
    key = sched["ntot"]
    if key not in _CACHE:
        _CACHE[key] = build(sched)
    nc = _CACHE[key]
    res = bass_utils.run_bass_kernel_spmd(nc, in_maps, core_ids=list(range(NC)))
    outs = []
    for c in range(NC):
        oc = res.results[c]["out"].reshape(128, W, F2)
        outs.append(oc.transpose(1, 0, 2).reshape(SLOTS, F2)[:NPC])
    return np.concatenate(outs, axis=0).astype(np.float32)


# revision 11
# speedup vs baseline: 1.1941x; 1.1941x over previous
"""2-layer GCN (GCNConv x2) on 8 TRN2 NeuronCores.

Strategy: partition destination nodes across cores (12500/core, padded to
12544 = 98 windows x 128 lanes). Per layer: local X@W matmul -> pre-scale by
D^-1/2 -> AllGather a bf16 message table -> per-edge gather via dma_gather
(int16 idx, 4 residue banks of 64B-strided rows) -> one-hot scatter matmul
into persistent PSUM window accumulators -> epilogue (scale, bias, act).
norm factors: out[d] = dis[d] * sum_e dis[src] * msg[src]  (self-loops are
plain edges), so no per-edge norm work is needed.
"""
import sys
import numpy as np
import ml_dtypes

from concourse import bass, mybir, tile, bacc, bass_utils
from concourse.library_config import mlp
from concourse.bass import exact_div

NC = 8
NN = 100000
NPC = 12500
SLOTS = 12544          # 98 windows * 128 lanes
W = 98
P = 128
F0, F1, F2 = 128, 16, 8
NQ = 4                 # residue banks (table row = 64B, stride = 256B)
MAXT_CALL = 31         # tiles per dma_gather call (<= 3968 idx, ring limit)

BF16 = mybir.dt.bfloat16
F32 = mybir.dt.float32
I16 = mybir.dt.int16


def _host_prep(x, edge_index, W1, b1, W2, b2):
    """Shard + schedule. Returns in_maps (list of dicts) and schedule."""
    s = edge_index[0].astype(np.int64)
    d = edge_index[1].astype(np.int64)
    loops = np.arange(NN, dtype=np.int64)
    s = np.concatenate([s, loops])
    d = np.concatenate([d, loops])

    deg = np.bincount(d, minlength=NN).astype(np.float32)  # includes loops

    cd = d // NPC
    jd = d - cd * NPC
    wd = jd >> 7
    pd = jd & 127
    cs = s // NPC
    js = s - cs * NPC
    g = cs * SLOTS + (js & 127) * W + (js >> 7)
    q = g & (NQ - 1)
    idx16 = (g >> 2).astype(np.int16)

    # group key: (core, q, w)
    key = (cd * NQ + q) * W + wd
    order = np.argsort(key, kind="stable")
    key_s = key[order]
    idx16_s = idx16[order]
    pd_s = pd[order]

    counts = np.bincount(key, minlength=NC * NQ * W).reshape(NC, NQ, W)
    T = np.maximum(1, -(-counts.max(axis=0) // 128))      # [NQ, W] tiles
    seg_slots = T * 128
    seg_start = np.concatenate([[0], np.cumsum(seg_slots.ravel())]).astype(np.int64)
    ntot = int(seg_start[-1])

    # within-group rank of each sorted edge
    grp_first = np.concatenate([[0], np.cumsum(np.bincount(key_s, minlength=NC * NQ * W))])
    rank = np.arange(len(key_s)) - grp_first[key_s]
    core_of = key_s // (NQ * W)
    seg_of = key_s % (NQ * W)
    pos = seg_start[seg_of] + rank

    idxflat = np.zeros((NC, ntot), dtype=np.int16)
    dlflat = np.full((NC, ntot), -1.0, dtype=np.float32)
    idxflat[core_of, pos] = idx16_s
    dlflat[core_of, pos] = pd_s.astype(np.float32)

    # images
    nt16 = ntot // 16
    ntt = ntot // 128
    idximg = np.zeros((NC, 128, nt16), dtype=np.int16)
    dlimg = np.zeros((NC, 128, ntt), dtype=np.float32)
    for c in range(NC):
        w16 = np.ascontiguousarray(idxflat[c].reshape(nt16, 16).T)
        for r in range(2 * NQ):   # replicate for queue core pairs 0..NQ-1
            idximg[c, r * 16:(r + 1) * 16] = w16
        dlimg[c] = dlflat[c].reshape(ntt, 128).T

    # per-tile (w, start, stop) + call schedule (within one q)
    tile_w, tile_st, tile_sp, calls = [], [], [], []
    gt = 0
    for qq in range(NQ):
        q_start = gt
        for ww in range(W):
            t = int(T[qq, ww])
            for k in range(t):
                tile_w.append(ww)
                tile_st.append(qq == 0 and k == 0)
                tile_sp.append(qq == NQ - 1 and k == t - 1)
            gt += t
        # split this q's tile range into calls
        a = q_start
        while a < gt:
            n = min(MAXT_CALL, gt - a)
            calls.append((a, n, qq))
            a += n
    ntiles = gt
    assert ntiles * 128 == ntot

    # dis arrays [128, 98] (exact f32 on host)
    dis_full = (1.0 / np.sqrt(deg)).astype(np.float32)
    degs = []
    for c in range(NC):
        dc = np.ones(SLOTS, dtype=np.float32)
        dc[:NPC] = dis_full[c * NPC:(c + 1) * NPC]
        degs.append(np.ascontiguousarray(dc.reshape(W, 128).T))

    in_maps = []
    for c in range(NC):
        xt = np.zeros((128, SLOTS), dtype=np.float32)
        xt[:, :NPC] = x[c * NPC:(c + 1) * NPC].T
        in_maps.append({
            "xT": xt,
            "idximg": idximg[c],
            "dlimg": dlimg[c],
            "dis": degs[c],
            "W1": W1.astype(np.float32),
            "W2": W2.astype(np.float32),
            "b1rep": np.tile(b1.astype(np.float32)[None, :], (128, 1)),
            "b2rep": np.tile(b2.astype(np.float32)[None, :], (128, 1)),
            "iota": np.tile(np.arange(128, dtype=ml_dtypes.bfloat16)[None, :], (128, 1)),
        })
    sched = dict(ntot=ntot, ntiles=ntiles, tile_w=tile_w, tile_st=tile_st,
                 tile_sp=tile_sp, calls=calls)
    return in_maps, sched


def _gather(gp, out_ap, in_ap, idxs_ap, num_idxs, elem_size, elem_step, qnum=0):
    stride_bytes = elem_step * mybir.dt.size(in_ap.dtype)
    return gp.add_instruction(mybir.InstDMAGatherAnt(
        name=gp.bass.get_next_instruction_name(),
        ins=[*gp.lower_ap_dma(in_ap, for_custom_bir_dma=True), gp.lower_ap(idxs_ap),
             gp.lower_val_access(gp.to_reg(num_idxs))],
        outs=[gp.lower_ap(out_ap)],
        transpose=False, num_idxs=num_idxs, elem_size=elem_size,
        stride_bytes_256=exact_div(stride_bytes, 256), gen_mode=0,
        single_packet=False, queue_num=qnum, sbuf_tokens_per_rank=0,
        sbuf_free_dim_per_rank=0, sbuf_free_dim_pad_per_rank=0, sbuf_byte_offset=0))


def build(sched, n_queues=2):
    from concourse.masks import make_identity
    nc = bacc.Bacc("TRN2", target_bir_lowering=False, debug=False, num_devices=NC,
                   num_swdge_queues=n_queues)
    ntot, ntiles = sched["ntot"], sched["ntiles"]
    tile_w, tile_st, tile_sp = sched["tile_w"], sched["tile_st"], sched["tile_sp"]
    calls = sched["calls"]

    xT_d = nc.dram_tensor("xT", [128, SLOTS], F32, kind="ExternalInput")
    idx_d = nc.dram_tensor("idximg", [128, ntot // 16], I16, kind="ExternalInput")
    dl_d = nc.dram_tensor("dlimg", [128, ntot // 128], F32, kind="ExternalInput")
    dis_d = nc.dram_tensor("dis", [128, W], F32, kind="ExternalInput")
    W1_d = nc.dram_tensor("W1", [F0, F1], F32, kind="ExternalInput")
    W2_d = nc.dram_tensor("W2", [F1, F2], F32, kind="ExternalInput")
    b1_d = nc.dram_tensor("b1rep", [128, F1], F32, kind="ExternalInput")
    b2_d = nc.dram_tensor("b2rep", [128, F2], F32, kind="ExternalInput")
    io_d = nc.dram_tensor("iota", [128, 128], BF16, kind="ExternalInput")
    out_d = nc.dram_tensor("out", [128, W * F2], F32, kind="ExternalOutput")

    with tile.TileContext(nc) as tc:
        nc.gpsimd.load_library(mlp)
        with (tc.tile_pool(name="sb", bufs=1) as sb,
              tc.tile_pool(name="gp", bufs=3) as gpool,
              tc.tile_pool(name="hp", bufs=4) as hpool,
              tc.tile_pool(name="ep", bufs=2) as epool,
              tc.tile_pool(name="ps", bufs=1, space="PSUM") as psp,
              tc.tile_pool(name="psm", bufs=1, space="PSUM") as psm,
              tc.tile_pool(name="dr", bufs=1, space="DRAM") as dr):
            # ---- static loads
            xT = sb.tile([128, SLOTS], F32)
            nc.sync.dma_start(out=xT[:], in_=xT_d.ap())
            IDX = sb.tile([128, ntot // 16], I16)
            nc.sync.dma_start(out=IDX[:], in_=idx_d.ap())
            DL = sb.tile([128, ntot // 128], F32)
            nc.sync.dma_start(out=DL[:], in_=dl_d.ap())
            DIS = sb.tile([128, W], F32)
            nc.sync.dma_start(out=DIS[:], in_=dis_d.ap())
            W1s = sb.tile([F0, F1], F32)
            nc.sync.dma_start(out=W1s[:], in_=W1_d.ap())
            W2s = sb.tile([F1, F2], F32)
            nc.sync.dma_start(out=W2s[:], in_=W2_d.ap())
            B1 = sb.tile([128, F1], F32)
            nc.sync.dma_start(out=B1[:], in_=b1_d.ap())
            B2 = sb.tile([128, F2], F32)
            nc.sync.dma_start(out=B2[:], in_=b2_d.ap())
            IOTA = sb.tile([128, 128], BF16)
            nc.sync.dma_start(out=IOTA[:], in_=io_d.ap())
            IDENT = sb.tile([128, 128], F32)
            make_identity(nc, IDENT[:])


            STG = [sb.tile([128, W * 32], BF16, tag=f"stg{i}", name=f"stg{i}") for i in range(2)]
            OUT = sb.tile([128, W * F2], F32)

            def edge_phase(layer):
                """gather from tbl{layer}, scatter into PSUM slots, epilogue."""
                tbl = dr.tile([NC * 128, W * 32], BF16, tag=f"tbl{layer}")
                bnc = dr.tile([128, W * 32], BF16, tag=f"bnc{layer}")
                nc.sync.dma_start(out=bnc[:], in_=STG[layer][:])
                cc = nc.gpsimd.collective_compute(
                    "AllGather", mybir.AluOpType.bypass,
                    replica_groups=[list(range(NC))],
                    ins=[bnc[:].opt()], outs=[tbl[:].opt()])
                tflat = tbl[:].rearrange("a b -> (a b)").rearrange(
                    "(m q e) -> q m e", q=NQ, e=32)
                ps = [psp.tile([128, 512], F32, tag=f"ps{i}", name=f"ps{i}") for i in range(4)]
                for t_ in ps:
                    nc.vector.memset(t_[:], 0.0)

                def pslot(w):
                    return ps[w // 32][:, (w % 32) * 16:(w % 32) * 16 + 16]

                from concourse.tile_rust import add_dep_helper
                for ci, (t0, nt, qq) in enumerate(calls):
                    g = gpool.tile([128, MAXT_CALL, 32], BF16, tag="g")
                    gi = _gather(nc.gpsimd, g[:, 0:nt, :], tflat[qq],
                                 IDX[:, t0 * 8:(t0 + nt) * 8], nt * 128, 32, 128,
                                 qnum=ci % 2)
                    add_dep_helper(gi.ins, cc.ins, sync=True,
                                   reason="gather reads allgathered table")
                    for k in range(nt):
                        t = t0 + k
                        ww = tile_w[t]
                        H = hpool.tile([128, 128], BF16, tag="h")
                        nc.vector.tensor_scalar(
                            out=H[:], in0=IOTA[:], scalar1=DL[:, t:t + 1],
                            scalar2=None, op0=mybir.AluOpType.is_equal)
                        nc.tensor.matmul(out=pslot(ww), lhsT=H[:],
                                         rhs=g[:, k, 0:16],
                                         start=False, stop=tile_sp[t],
                                         skip_group_check=True)
                        if tile_sp[t]:
                            dis_w = DIS[:, ww:ww + 1]
                            if layer == 0:
                                t1 = epool.tile([128, F1], F32, tag="t1")
                                nc.vector.tensor_scalar(out=t1[:], in0=pslot(ww),
                                                        scalar1=dis_w, scalar2=None,
                                                        op0=mybir.AluOpType.mult)
                                t2 = epool.tile([128, F1], F32, tag="t2")
                                nc.vector.tensor_tensor(out=t2[:], in0=t1[:], in1=B1[:],
                                                        op=mybir.AluOpType.add)
                                t3 = epool.tile([128, F1], F32, tag="t3")
                                nc.scalar.activation(t3[:], t2[:],
                                                     mybir.ActivationFunctionType.Relu)
                                nc.vector.tensor_scalar(
                                    out=STG[1][:, ww * 32:ww * 32 + 16], in0=t3[:],
                                    scalar1=dis_w, scalar2=None,
                                    op0=mybir.AluOpType.mult)
                            else:
                                y = epool.tile([128, F1], F32, tag="y")
                                nc.vector.tensor_scalar(out=y[:], in0=pslot(ww),
                                                        scalar1=dis_w, scalar2=None,
                                                        op0=mybir.AluOpType.mult)
                                pt = psm.tile([F1, 128], F32, tag="pt")
                                nc.tensor.transpose(out=pt[:], in_=y[:],
                                                    identity=IDENT[:])
                                s2t = epool.tile([F1, 128], F32, tag="s2t")
                                nc.scalar.copy(s2t[:], pt[:])
                                po = psm.tile([128, F2], F32, tag="po")
                                nc.tensor.matmul(out=po[:], lhsT=s2t[:], rhs=W2s[:],
                                                 start=True, stop=True)
                                o1 = epool.tile([128, F2], F32, tag="o1")
                                nc.vector.tensor_tensor(out=o1[:], in0=po[:], in1=B2[:],
                                                        op=mybir.AluOpType.add)
                                nc.scalar.activation(
                                    OUT[:, ww * F2:(ww + 1) * F2], o1[:],
                                    mybir.ActivationFunctionType.Sigmoid)

            # ---- layer 1 local matmul + scale -> staging table
            nc.vector.memset(STG[0][:], 0.0)
            nc.vector.memset(STG[1][:], 0.0)
            for w in range(W):
                pm = psm.tile([128, F1], F32, tag="mm1")
                nc.tensor.matmul(out=pm[:], lhsT=xT[:, w * 128:(w + 1) * 128],
                                 rhs=W1s[:], start=True, stop=True)
                nc.vector.tensor_scalar(out=STG[0][:, w * 32:w * 32 + 16], in0=pm[:],
                                        scalar1=DIS[:, w:w + 1], scalar2=None,
                                        op0=mybir.AluOpType.mult)
            edge_phase(0)
            edge_phase(1)
            nc.sync.dma_start(out=out_d.ap(), in_=OUT[:])
    nc.compile()
    return nc


_CACHE = {}


def kernel(x, edge_index, W1, b1, W2, b2):
    x = np.asarray(x, dtype=np.float32)
    edge_index = np.asarray(edge_index)
    in_maps, sched = _host_prep(x, edge_index, np.asarray(W1), np.asarray(b1),
                                np.asarray(W2), np.asarray(b2))
    key = sched["ntot"]
    if key not in _CACHE:
        _CACHE[key] = build(sched)
    nc = _CACHE[key]
    res = bass_utils.run_bass_kernel_spmd(nc, in_maps, core_ids=list(range(NC)))
    outs = []
    for c in range(NC):
        oc = res.results[c]["out"].reshape(128, W, F2)
        outs.append(oc.transpose(1, 0, 2).reshape(SLOTS, F2)[:NPC])
    return np.concatenate(outs, axis=0).astype(np.float32)
